# revision 22
# baseline (speedup 1.0000x reference)
"""Trainium2 Bass kernel for the pairwise-minimum-distance loss.

Math: for each frame b (bs*seq flattened) and articulator pair (i, j),
  min_dists[b,i,j] = min_{s,t} ||p_{b,i,s} - p_{b,j,t}||
loss = mean(masks * min_dists).

Key identities exploited:
 - sqrt is monotone: min sqrt(relu(d2)) = sqrt(relu(min d2)) -> only the
   10x10 minima need sqrt (host side), not 128M distances.
 - min_dists is symmetric with zero diagonal -> only pairs i<j computed.

d2 for a whole frame row-chunk comes from one K=4 matmul:
  lhsT rows = [x_i(s), y_i(s), |p_i(s)|^2, 1]        (K=4, M=100)
  rhs  cols = [-2x_j(t), -2y_j(t), 1, |q_j(t)|^2]    (K=4, N<=450)
  out[(i,s),(j,t)] = sq_i + sq_j - 2(x x' + y y') = d2   (full fp32)

Four frames are processed concurrently on the PE via row tiling
(tile_position=(32q,0), q=0..3), each writing a different PSUM bank;
DVE then does one blocked min-reduce over t per 4-frame round.
min over s + sqrt + mask + mean happen on host (the data-parallel
"all-reduce": 512 frames -> 64 per core across 8 cores).
"""

import sys

if "/opt/trn_rl_repo" not in sys.path:
    sys.path.insert(0, "/opt/trn_rl_repo")

import numpy as np

import concourse.bass as bass
import concourse.mybir as mybir
import concourse.tile as tile
from concourse.bass_utils import run_bass_kernel_spmd

# Problem geometry (hardcoded per harness contract)
BS, SEQ, N_ART, N_DIM, N_SAMP = 4, 128, 10, 2, 50
BT = BS * SEQ            # 512 frames
N_CORES = 8
FPC = BT // N_CORES      # 64 frames per core
ROWS = N_ART * N_SAMP    # 500 (i,s) rows per frame

NQ = 4                   # concurrent PE row-groups (frames per round)
FPQ = FPC // NQ          # 16 frames per row-group
QCOLS = 2 * FPQ * ROWS   # free cols per row-group: lhs half | rhs half

# frame rows (i,s) flattened to 500, cut into 128-row matmul chunks;
# chunk k needs cols j >= jmin_k where jmin_k = (lowest i in chunk)+1
N_CHUNKS = 4
CHUNK_LO = [0, 128, 256, 384]
CHUNK_M = [128, 128, 128, 116]
CHUNK_JMIN = [lo // N_SAMP + 1 for lo in CHUNK_LO]          # [1, 3, 6, 8]
CHUNK_NJ = [N_ART - jm for jm in CHUNK_JMIN]                # [9, 7, 4, 2]
CHUNK_OFF = [0, 9, 16, 20]  # running offsets into the 22 (c,j) slots
TRI = sum(CHUNK_NJ)      # 22

_NC_CACHE = {}


def _build_nc():
    f32 = mybir.dt.float32
    nc = bass.Bass()
    ops_d = nc.declare_dram_parameter("ops", [4 * NQ, QCOLS], f32, isOutput=False)
    out_d = nc.declare_dram_parameter("rowmin", [128, FPC * TRI], f32, isOutput=True)

    with tile.TileContext(nc) as tc:
        with (
            tc.tile_pool(name="ops", bufs=1) as ops_pool,
            tc.tile_pool(name="res", bufs=1) as res_pool,
            tc.tile_pool(name="ps", bufs=2, space="PSUM") as ps_pool,
        ):
            t = ops_pool.tile([128, QCOLS], f32, tag="ops")
            r_sb = res_pool.tile([128, FPC * TRI], f32)

            # operands live on partitions 32q..32q+3 only
            for q in range(NQ):
                nc.gpsimd.dma_start(
                    t[32 * q : 32 * q + 4, :], ops_d[4 * q : 4 * q + 4, :]
                )

            for f in range(FPQ):
                for c in range(N_CHUNKS):
                    nj = CHUNK_NJ[c]
                    m = CHUNK_M[c]
                    ncols = nj * N_SAMP
                    ps = ps_pool.tile([128, NQ * 512], f32)
                    for q in range(NQ):
                        lo = f * ROWS
                        nc.tensor.matmul(
                            ps[0:m, q * 512 : q * 512 + ncols],
                            t[32 * q : 32 * q + 4, lo + CHUNK_LO[c] : lo + CHUNK_LO[c] + m],
                            t[32 * q : 32 * q + 4, FPQ * ROWS + lo + CHUNK_JMIN[c] * N_SAMP : FPQ * ROWS + lo + ROWS],
                            start=True,
                            stop=True,
                            tile_position=(32 * q, 0),
                        )
                    # one blocked min over t for all 4 concurrent frames
                    in_ap = (
                        ps[0:m, :]
                        .rearrange("p (q x) -> p q x", q=NQ)[:, :, 0:ncols]
                        .rearrange("p q (j s) -> p q j s", s=N_SAMP)
                    )
                    out_ap = (
                        r_sb[0:m, :]
                        .rearrange("p (q x) -> p q x", q=NQ)[
                            :, :, f * TRI + CHUNK_OFF[c] : f * TRI + CHUNK_OFF[c] + nj
                        ]
                    )
                    nc.vector.tensor_reduce(
                        out=out_ap,
                        in_=in_ap,
                        axis=mybir.AxisListType.X,
                        op=mybir.AluOpType.min,
                    )

            nc.sync.dma_start(out_d[:], r_sb[:])

    _prune_redundant_waits(nc)
    return nc


def _prune_redundant_waits(nc):
    """Remove semaphore waits that are already guaranteed.

    Walrus's per-instruction sync encoding has very few wait slots (a
    matmul's LDWEIGHTS takes one), so Tile's conservatively emitted waits
    can fail codegen with "Too many sync wait commands". Two sources of
    redundancy are provable from the scheduled program:
      * same-engine ordering: engines execute and retire their stream in
        program order, so a wait on the engine's own semaphore is free;
      * transitivity: if X waits on DVE>=80 and the 80th DVE op itself
        waited on PE>=320, then X's PE>=320 wait is implied.
    Completion-order assumptions: compute engines retire in order; DMA
    completions are in order per queue semaphore (which is how Tile
    assigns them).
    """
    insts = []
    for blk in nc.m.functions[0].blocks:
        insts.extend(blk.instructions)

    def is_async(inst):
        # DMA transfers signal completion from the DMA queue, not the
        # issuing engine; their sems are DMASW*/DMAHW* named.
        si = inst.sync_info
        if not si:
            return False
        return any("DMA" in (u.ant_name or "") for u in si.on_update)

    # engines with strict in-order retirement (POOL's 8 Q7 cores are not)
    ORDERED = ("PE", "DVE", "ACT", "SP", "Activation", "Vector", "Tensor", "Sync")

    # per-semaphore ordered updater list with cumulative values
    sem_updaters = {}   # sem id -> list[(inst_idx, cum_value)]
    sem_prev = {}       # inst_idx -> previous updater of the same sem (queue FIFO)
    eng_prev = {}       # inst_idx -> same-engine previous in-order inst_idx
    last_on_engine = {}
    for ix, inst in enumerate(insts):
        eng = str(inst.engine)
        asy = is_async(inst)
        if not asy and any(k in eng for k in ORDERED):
            if eng in last_on_engine:
                eng_prev[ix] = last_on_engine[eng]
            last_on_engine[eng] = ix
        si = inst.sync_info
        if not si:
            continue
        for u in si.on_update:
            if u.update_mode not in ("sem-inc", "sem-add-imm") or u.update_value is None:
                continue
            lst = sem_updaters.setdefault(u.id, [])
            if asy and lst:
                sem_prev[ix] = lst[-1][0]
            cum = (lst[-1][1] if lst else 0) + u.update_value
            lst.append((ix, cum))

    def updater_for(sem_id, value):
        lst = sem_updaters.get(sem_id)
        if not lst:
            return None
        for ix, cum in lst:
            if cum >= value:
                return ix
        return None

    # preds_of(ix) = instruction indices whose *completion* is implied
    # before ix's effects happen
    def preds_of(ix):
        out = []
        if ix in eng_prev:
            out.append(eng_prev[ix])
        if ix in sem_prev:
            out.append(sem_prev[ix])
        si = insts[ix].sync_info
        if si:
            for w in si.on_wait:
                if w.wait_mode != "sem-ge-imm" or w.wait_value is None:
                    continue
                up = updater_for(w.id, w.wait_value)
                if up is not None:
                    out.append(up)
        return out

    for ix, inst in enumerate(insts):
        si = inst.sync_info
        if not si or len(si.on_wait) <= 1:
            continue
        keep = list(si.on_wait)
        changed = True
        while changed and len(keep) > 1:
            changed = False
            for w in keep:
                if w.wait_mode != "sem-ge-imm" or w.wait_value is None:
                    continue
                up = updater_for(w.id, w.wait_value)
                if up is None:
                    continue
                # closure from engine-pred + other kept waits
                result = set()
                stack = []
                if ix in eng_prev:
                    stack.append(eng_prev[ix])
                for w2 in keep:
                    if w2 is w:
                        continue
                    if w2.wait_mode != "sem-ge-imm" or w2.wait_value is None:
                        continue
                    u2 = updater_for(w2.id, w2.wait_value)
                    if u2 is not None:
                        stack.append(u2)
                while stack:
                    p = stack.pop()
                    if p in result:
                        continue
                    result.add(p)
                    stack.extend(preds_of(p))
                if up in result:
                    keep.remove(w)
                    changed = True
                    break
        if len(keep) < len(si.on_wait):
            inst.sync_info = type(si)(on_wait=keep, on_update=si.on_update)


def _get_nc():
    if "nc" not in _NC_CACHE:
        _NC_CACHE["nc"] = _build_nc()
    return _NC_CACHE["nc"]


def _make_in_maps(outputs):
    pts = outputs.reshape(BT, N_ART, N_DIM, N_SAMP)
    x = pts[:, :, 0, :]                      # (BT, N_ART, N_SAMP)
    y = pts[:, :, 1, :]
    sq = x * x + y * y
    ones = np.ones_like(x)

    lhs_all = np.stack([x, y, sq, ones], axis=0).reshape(4, BT, ROWS)
    rhs_all = np.stack([-2.0 * x, -2.0 * y, ones, sq], axis=0).reshape(4, BT, ROWS)

    in_maps = []
    for k in range(N_CORES):
        sl = slice(k * FPC, (k + 1) * FPC)
        # frames of this core -> (NQ, FPQ): row-group q holds frames q*FPQ..q*FPQ+FPQ-1
        lg = lhs_all[:, sl].reshape(4, NQ, FPQ * ROWS)
        rg = rhs_all[:, sl].reshape(4, NQ, FPQ * ROWS)
        # ops[(q,k), (lhs|rhs)]: (NQ, 4, 2*FPQ*ROWS) -> (4*NQ, QCOLS)
        ops = np.concatenate([lg, rg], axis=2).transpose(1, 0, 2).reshape(4 * NQ, QCOLS)
        in_maps.append({"ops": np.ascontiguousarray(ops)})
    return in_maps


def kernel(outputs, masks):
    outputs = np.asarray(outputs, dtype=np.float32)
    masks = np.asarray(masks, dtype=np.float32)
    in_maps = _make_in_maps(outputs)

    nc = _get_nc()
    try:
        res = run_bass_kernel_spmd(nc, in_maps, list(range(N_CORES)))
    except Exception:
        # transient device states (e.g. a wedged exec unit from a prior
        # failed load) usually clear on retry
        res = run_bass_kernel_spmd(nc, in_maps, list(range(N_CORES)))

    md2 = np.full((BT, N_ART, N_ART), np.inf, dtype=np.float32)
    for k in range(N_CORES):
        r = np.asarray(res.results[k]["rowmin"])          # (128, FPC*TRI)
        r = r.reshape(128, NQ, FPQ, TRI)
        sl = slice(k * FPC, (k + 1) * FPC)
        for c in range(N_CHUNKS):
            m, jmin, nj = CHUNK_M[c], CHUNK_JMIN[c], CHUNK_NJ[c]
            blk = r[:m, :, :, CHUNK_OFF[c] : CHUNK_OFF[c] + nj]  # (m, NQ, FPQ, nj)
            gi = CHUNK_LO[c] + np.arange(m)
            i_of = gi // N_SAMP
            for i in np.unique(i_of):
                part = blk[i_of == i].min(axis=0)          # (NQ, FPQ, nj)
                for jj in range(nj):
                    j = jmin + jj
                    if j > i:
                        cur = md2[sl, i, j]
                        md2[sl, i, j] = np.minimum(cur, part[:, :, jj].reshape(FPC))

    iu, ju = np.triu_indices(N_ART, k=1)
    md = np.zeros((BT, N_ART, N_ART), dtype=np.float32)
    md[:, iu, ju] = np.sqrt(np.maximum(md2[:, iu, ju], 0.0))
    md = md + md.transpose(0, 2, 1)   # mirror; diagonal stays 0
    loss = np.mean(masks.reshape(BT, N_ART, N_ART) * md)
    return np.float32(loss)
